# revision 2
# baseline (speedup 1.0000x reference)
"""AveragePrevEmbeddingsLM Trainium2 kernel (8 NeuronCores, vocab-sharded).

logits[b, t, v] = mean(emb_table[x[b, :t+1]]) @ W.T + b_vec

v2 design (vs v1 baseline at ~235 us):
  - emb table host-prepped as bf16 with each row doubled: emb2[V, 128] =
    [row, row]. The gather lands [128tok, 128] tiles whose two 64-col
    halves are identical.
  - "Transpose" per 128-token block is a regular matmul with identity
    moving operand: psum_t[:, blk] = gath_blk.T @ I. Output is
    [128 emb-dup, 128 tok] f32 in one PSUM bank per 512-token quarter.
    Both 64-row strips of seg are produced at once (no replicate DMA),
    and the PE stays in (128,128) tile mode (no transpose-mode switch).
  - One DVE tensor_tensor_scan per quarter reads PSUM directly and
    writes seg bf16 [128, 512] (prefix sums), carry chained via
    initial=prev_seg[:, -1:]. This kills v1's per-block PSUM->SBUF
    copies and the separate f32->bf16 cast.
  - Projection: row-tiled matmul pairs (strips at partitions 0:64 /
    64:128, auto tile_position (0,0)/(64,0)) into 3x [128,2,512] PSUM
    pair-tiles. W host-prepped bf16, duplicated across both strips.
  - Drains (PSUM f32 -> SBUF fp8, scaled by 1/(t+1) * g(t)): ACT takes
    9/16 of pairs, DVE 7/16 (matches 1.2 vs 0.96 GHz + DVE scan load).
  - One contiguous 512KB out-DMA per m-tile (4000B rows).
  - fp8(e3m4) output with per-position power-of-2 normalizer g(t); host
    divides g back out and adds the exact f32 bias (bias never on device).
"""

import os
import sys

import numpy as np

for _p in ("/opt/trn_rl_repo",):
    if _p not in sys.path and os.path.isdir(_p):
        sys.path.append(_p)

VOCAB, EMB, B, SEQ = 32000, 64, 4, 2048
NCORES = 8
VS = VOCAB // NCORES       # vocab shard per core (4000)
TOK = B * SEQ              # 8192
BLK = SEQ // 128           # 16 blocks per batch row
MTILES = TOK // 128        # 64
NCHUNK = 8
CHUNK = VS // NCHUNK       # 500
QT = 4                     # m-tiles per quarter
NQ = MTILES // QT          # 16 quarters
QSEQ = QT * 128            # 512

_prog_cache = {}


def _g_scale():
    """Per-position power-of-2 normalizer: the pooled (pre-bias) logit at
    position t has std ~0.577/sqrt(t+1); scale it to ~unit range so the
    e3m4 output stays in the normal range. Host divides it back out."""
    t = np.arange(SEQ, dtype=np.float64)
    return (2.0 ** np.round(np.log2(np.sqrt(t + 1) / 0.577))).astype(np.float32)


def _build():
    from concourse import bacc
    import concourse.mybir as mybir
    import concourse.tile as tile
    import concourse.bass as bass

    f32 = mybir.dt.float32
    bf16 = mybir.dt.bfloat16
    fp8 = mybir.dt.float8e3
    i32 = mybir.dt.int32

    nc = bacc.Bacc(None, target_bir_lowering=False)

    emb2_d = nc.dram_tensor("emb2", [VOCAB, 128], bf16, kind="ExternalInput")
    idx_d = nc.dram_tensor("idx", [128, MTILES], i32, kind="ExternalInput")
    wtb_d = nc.dram_tensor("wtb", [128, NCHUNK, CHUNK], bf16, kind="ExternalInput")
    ident_d = nc.dram_tensor("ident", [128, 128], bf16, kind="ExternalInput")
    recip_d = nc.dram_tensor("recip", [128, BLK], f32, kind="ExternalInput")
    out_d = nc.dram_tensor("out", [TOK, VS], fp8, kind="ExternalOutput")

    with tile.TileContext(nc) as tc:
        with (
            tc.tile_pool(name="const", bufs=1) as constp,
            tc.tile_pool(name="gath", bufs=3) as gathp,
            tc.tile_pool(name="seg", bufs=2) as segp,
            tc.tile_pool(name="outp", bufs=3) as outp,
            tc.tile_pool(name="ptp", bufs=2, space="PSUM") as ptp,
            tc.tile_pool(name="pmm", bufs=3, space="PSUM") as pmmp,
        ):
            wtb_sb = constp.tile([128, NCHUNK, CHUNK], bf16)
            nc.sync.dma_start(wtb_sb[:], wtb_d[:])
            recip_sb = constp.tile([128, BLK], f32)
            nc.sync.dma_start(recip_sb[:], recip_d[:])
            idx_sb = constp.tile([128, MTILES], i32)
            nc.sync.dma_start(idx_sb[:], idx_d[:])
            ident_sb = constp.tile([128, 128], bf16)
            nc.sync.dma_start(ident_sb[:], ident_d[:])
            # bypassed data1 operand for the scan (values never used)
            dummy_sb = constp.tile([128, QSEQ], bf16)
            nc.vector.memset(dummy_sb[:], 0.0)

            gath_of = {}
            seg_of = {}
            seg_prev = [None]

            def head(Q):
                g = gathp.tile([128, QT, 128], bf16, tag="gath", name="gath")
                for i in range(QT):
                    m = Q * QT + i
                    nc.gpsimd.indirect_dma_start(
                        out=g[:, i, :],
                        out_offset=None,
                        in_=emb2_d[:],
                        in_offset=bass.IndirectOffsetOnAxis(
                            ap=idx_sb[:, m:m + 1], axis=0,
                        ),
                    )
                gath_of[Q] = g

            def prep(Q):
                q = Q % (BLK // QT)
                g = gath_of.pop(Q)
                pt = ptp.tile([128, QSEQ], f32, tag="pt", name="pt")
                for i in range(QT):
                    nc.tensor.matmul(
                        pt[:, i * 128:(i + 1) * 128],
                        g[:, i, :],
                        ident_sb[:],
                        start=True,
                        stop=True,
                    )
                seg = segp.tile([128, QSEQ], bf16, tag="seg", name="seg")
                initial = (0.0 if q == 0 else
                           seg_prev[0][:, QSEQ - 1:QSEQ])
                nc.vector.tensor_tensor_scan(
                    seg[:],
                    pt[:],
                    dummy_sb[:],
                    initial,
                    op0=mybir.AluOpType.add,
                    op1=mybir.AluOpType.bypass,
                )
                seg_of[Q] = seg
                seg_prev[0] = seg

            def proj(Q):
                q = Q % (BLK // QT)
                seg = seg_of.pop(Q)
                for i in range(QT):
                    m = Q * QT + i
                    mb = q * QT + i
                    otile = outp.tile([128, NCHUNK, CHUNK], fp8, tag="ot",
                                      name="ot")
                    scale = recip_sb[:, mb:mb + 1]
                    for pr in range(NCHUNK // 2):
                        ps = pmmp.tile([128, 2, 512], f32, tag="pmm",
                                       name="pmm")
                        for j in range(2):
                            nc.tensor.matmul(
                                ps[:, j, 0:CHUNK],
                                seg[64 * j:64 * j + 64,
                                    i * 128:(i + 1) * 128],
                                wtb_sb[64 * j:64 * j + 64, 2 * pr + j, :],
                                start=True,
                                stop=True,
                            )
                        osl = otile[:, 2 * pr:2 * pr + 2, :]
                        # ACT 9/16 of pair drains, DVE 7/16
                        use_act = pr in (0, 2) or (pr == 1 and m % 4 == 0)
                        if use_act:
                            nc.scalar.activation(
                                osl, ps[:, :, 0:CHUNK],
                                mybir.ActivationFunctionType.Copy,
                                scale=scale,
                            )
                        else:
                            nc.vector.tensor_scalar_mul(
                                osl, ps[:, :, 0:CHUNK], scale)
                    nc.sync.dma_start(
                        out_d[m * 128:(m + 1) * 128, :], otile[:])

            head(0)
            prep(0)
            head(1)
            for Q in range(NQ):
                if Q + 1 < NQ:
                    prep(Q + 1)
                if Q + 2 < NQ:
                    head(Q + 2)
                proj(Q)

    nc.compile()
    return nc


def _get_prog():
    if "v2" not in _prog_cache:
        _prog_cache["v2"] = _build()
    return _prog_cache["v2"]


def _make_in_maps(emb_table, W, b, x):
    import ml_dtypes
    bf16 = ml_dtypes.bfloat16

    emb_table = np.asarray(emb_table, dtype=np.float32)
    W = np.asarray(W, dtype=np.float32)
    x = np.asarray(x).astype(np.int64).reshape(B, SEQ)

    emb_bf = emb_table.astype(bf16)
    emb2 = np.ascontiguousarray(np.concatenate([emb_bf, emb_bf], axis=1))

    # idx layout: token m*128 + p -> idx[p, m]
    wrapped = np.ascontiguousarray(
        x.reshape(-1).reshape(MTILES, 128).T.astype(np.int32)
    )

    ident = np.ascontiguousarray(np.eye(128, dtype=bf16))

    i = np.arange(128)[:, None]
    mb = np.arange(BLK)[None, :]
    t = mb * 128 + i                                   # position (p, mb)
    recip = (1.0 / (t + 1)).astype(np.float32)
    g = _g_scale()
    recip = (recip * g[t.ravel()].reshape(t.shape)).astype(np.float32)

    in_maps = []
    for c in range(NCORES):
        wtb = np.zeros((128, VS), dtype=bf16)
        wt = W[c * VS:(c + 1) * VS, :].T.astype(bf16)
        wtb[0:EMB] = wt
        wtb[EMB:2 * EMB] = wt
        in_maps.append({
            "emb2": emb2,
            "idx": wrapped,
            "wtb": np.ascontiguousarray(wtb.reshape(128, NCHUNK, CHUNK)),
            "ident": ident,
            "recip": recip,
        })
    return in_maps


def kernel(emb_table, W, b, x, trace=False):
    from concourse.bass_utils import run_bass_kernel_spmd

    nc = _get_prog()
    in_maps = _make_in_maps(emb_table, W, b, x)
    res = run_bass_kernel_spmd(
        nc, in_maps, core_ids=list(range(NCORES)), trace=trace,
    )

    out = np.empty((TOK, VOCAB), dtype=np.float32)
    for c in range(NCORES):
        out[:, c * VS:(c + 1) * VS] = np.asarray(
            res.results[c]["out"]).astype(np.float32)
    out = out.reshape(B, SEQ, VOCAB)
    # dequant epilogue: undo the power-of-2 normalizer and add the exact
    # f32 bias (kept out of the quantized device output).
    inv_g = (1.0 / _g_scale()).astype(np.float32)
    out *= inv_g[None, :, None]
    out += np.asarray(b, dtype=np.float32)[None, None, :]
    if trace:
        return out, res
    return out
